# revision 2
# baseline (speedup 1.0000x reference)
"""Bass/Trainium2 kernel for nn_DiagWeightLayer: out = x * weight (column scale).

x: (32768, 1024) f32, weight: (1024,) f32.
Data-parallel over 8 NeuronCores: each core processes a (4096, 1024) row
shard of x; the weight vector is replicated to every core.

The problem is pure HBM traffic (memory regime), so x and out travel as
bfloat16: the host quantizes x (rel err <= 2^-8) and the device rounds the
product once more (<= 2^-8), ~8e-3 worst-case elementwise vs the 2e-2
gate. bf16 keeps f32's exponent range, so near-zero values keep full
relative accuracy under any error convention (no fp16 denormal cliff).
Halving the DMA bytes halves the bandwidth-bound runtime vs the f32
baseline (99.2us -> ~50us/core at the ~340 GB/s per-core HBM share).
"""

import time

import numpy as np

import concourse.bacc as bacc
import concourse.bass as bass
import concourse.tile as tile
from concourse import mybir
from concourse.bass_utils import run_bass_kernel_spmd

N_CORES = 8
ROWS, COLS = 32768, 1024
SHARD = ROWS // N_CORES  # 4096 rows per core
P = 128  # SBUF partitions
N_CHUNKS = SHARD // P  # 32 row-chunks of 128 rows

try:  # numpy bf16 via ml_dtypes (ships with jax)
    import ml_dtypes

    BF16 = np.dtype(ml_dtypes.bfloat16)
except Exception:  # pragma: no cover
    import jax.numpy as jnp

    BF16 = np.dtype(jnp.bfloat16)

_DT = {"bf16": mybir.dt.bfloat16, "f32": mybir.dt.float32, "f16": mybir.dt.float16}
_NP = {"bf16": BF16, "f32": np.dtype(np.float32), "f16": np.dtype(np.float16)}


def build(reps=1, blk=8, bufs=4, dtype="bf16", fused_mul=True, layout="pn",
          wmode="pbcast"):
    """Build the per-core Bass program.

    reps: repeat the whole compute (for wall-clock slope timing).
    blk: 128-row chunks packed per SBUF tile (one DMA each way per tile).
    bufs: tile-pool slots (pipeline depth).
    dtype: on-wire dtype for x and out ("bf16" halves HBM traffic vs "f32").
    fused_mul: one tensor_mul per tile with free-dim-broadcast weight
        instead of one tensor_mul per chunk.
    layout: "pn" = consecutive rows per partition (contiguous blk*row DMA
        descriptor per partition line); "np" = round-robin rows across
        partitions (one-row descriptors).
    wmode: "pbcast" = single-partition DMA + gpsimd partition_broadcast;
        "dma" = 128-descriptor broadcast DMA straight from DRAM.
    """
    assert N_CHUNKS % blk == 0
    n_tiles = N_CHUNKS // blk
    dt = _DT[dtype]
    nc = bacc.Bacc()
    x = nc.dram_tensor("x", [SHARD, COLS], dt, kind="ExternalInput")
    w = nc.dram_tensor("weight", [COLS], dt, kind="ExternalInput")
    out = nc.dram_tensor("out", [SHARD, COLS], dt, kind="ExternalOutput")

    # DRAM view: [partition, chunk, col].
    if layout == "pn":
        xv = x.rearrange("(p n) m -> p n m", p=P)
        ov = out.rearrange("(p n) m -> p n m", p=P)
    else:
        xv = x.rearrange("(n p) m -> p n m", p=P)
        ov = out.rearrange("(n p) m -> p n m", p=P)

    with tile.TileContext(nc) as tc:
        with (
            tc.tile_pool(name="singles", bufs=1) as singles,
            tc.tile_pool(name="xs", bufs=bufs) as xpool,
        ):
            # Replicate weight across all 128 partitions.
            w_sb = singles.tile([P, COLS], dt)
            if wmode == "pbcast":
                nc.sync.dma_start(out=w_sb[:1, :], in_=w[None, :])
                nc.gpsimd.partition_broadcast(w_sb[:], w_sb[:1, :])
            else:
                nc.sync.dma_start(
                    out=w_sb[:], in_=w[None, :].to_broadcast([P, COLS])
                )

            for _ in range(reps):
                for i in range(n_tiles):
                    xt = xpool.tile([P, blk, COLS], dt)
                    nc.sync.dma_start(
                        out=xt[:], in_=xv[:, i * blk : (i + 1) * blk, :]
                    )
                    if fused_mul:
                        nc.vector.tensor_mul(
                            xt[:], xt[:], w_sb[:, None, :].to_broadcast([P, blk, COLS])
                        )
                    else:
                        for j in range(blk):
                            nc.vector.tensor_mul(xt[:, j, :], xt[:, j, :], w_sb[:])
                    nc.sync.dma_start(
                        out=ov[:, i * blk : (i + 1) * blk, :], in_=xt[:]
                    )
    nc.finalize()
    return nc


def prep_in_maps(x: np.ndarray, weight: np.ndarray, dtype="bf16"):
    """Cast to the on-wire dtype and slice per-core row shards."""
    npdt = _NP[dtype]
    x = np.ascontiguousarray(x, dtype=np.float32).astype(npdt)
    w = np.ascontiguousarray(weight, dtype=np.float32).astype(npdt)
    return [
        {"x": x[i * SHARD : (i + 1) * SHARD], "weight": w} for i in range(N_CORES)
    ]


_nc_cache: dict = {}


def _get_nc(dtype="bf16"):
    if dtype not in _nc_cache:
        _nc_cache[dtype] = build(dtype=dtype)
    return _nc_cache[dtype]


def kernel(x: np.ndarray, weight: np.ndarray) -> np.ndarray:
    nc = _get_nc()
    in_maps = prep_in_maps(x, weight)
    # The device intermittently reports NRT_EXEC_UNIT_UNRECOVERABLE under
    # load (observed on idle-kernel runs too, not workload-dependent); it
    # clears on the next NEFF load, so retry once before giving up.
    last_err = None
    for attempt in range(2):
        try:
            res = run_bass_kernel_spmd(nc, in_maps, list(range(N_CORES))).results
            return np.concatenate([r["out"] for r in res], axis=0).astype(np.float32)
        except Exception as e:  # noqa: BLE001
            last_err = e
            time.sleep(2.0)
    raise last_err


# revision 6
# speedup vs baseline: 1.0382x; 1.0382x over previous
"""Bass/Trainium2 kernel for nn_DiagWeightLayer: out = x * weight (column scale).

x: (32768, 1024) f32, weight: (1024,) f32.
Data-parallel over 8 NeuronCores: each core processes a (4096, 1024) row
shard of x; the weight vector is replicated to every core.

The problem is pure HBM traffic (memory regime), so x and out travel as
bfloat16: the host quantizes x (rel err <= 2^-8) and the device rounds the
product once more (<= 2^-8), ~8e-3 worst-case elementwise vs the 2e-2
gate. bf16 keeps f32's exponent range, so near-zero values keep full
relative accuracy under any error convention (no fp16 denormal cliff).
Halving the DMA bytes halves the bandwidth-bound runtime vs the f32
baseline (99.2us -> ~50us/core at the ~340 GB/s per-core HBM share).
"""

import time

import numpy as np

import concourse.bacc as bacc
import concourse.bass as bass
import concourse.tile as tile
from concourse import mybir
from concourse.bass_utils import run_bass_kernel_spmd

N_CORES = 8
ROWS, COLS = 32768, 1024
SHARD = ROWS // N_CORES  # 4096 rows per core
P = 128  # SBUF partitions
N_CHUNKS = SHARD // P  # 32 row-chunks of 128 rows

try:  # numpy bf16 via ml_dtypes (ships with jax)
    import ml_dtypes

    BF16 = np.dtype(ml_dtypes.bfloat16)
except Exception:  # pragma: no cover
    import jax.numpy as jnp

    BF16 = np.dtype(jnp.bfloat16)

_DT = {"bf16": mybir.dt.bfloat16, "f32": mybir.dt.float32, "f16": mybir.dt.float16}
_NP = {"bf16": BF16, "f32": np.dtype(np.float32), "f16": np.dtype(np.float16)}


def build(reps=1, blk=16, bufs=3, dtype="bf16", fused_mul=True, layout="pn",
          wmode="pbcast", qsplit="none"):
    """Build the per-core Bass program.

    reps: repeat the whole compute (for wall-clock slope timing).
    blk: 128-row chunks packed per SBUF tile (one DMA each way per tile).
    bufs: tile-pool slots (pipeline depth).
    dtype: on-wire dtype for x and out ("bf16" halves HBM traffic vs "f32").
    fused_mul: one tensor_mul per tile with free-dim-broadcast weight
        instead of one tensor_mul per chunk.
    layout: "pn" = consecutive rows per partition (contiguous blk*row DMA
        descriptor per partition line); "np" = round-robin rows across
        partitions (one-row descriptors).
    wmode: "pbcast" = single-partition DMA + gpsimd partition_broadcast;
        "dma" = 128-descriptor broadcast DMA straight from DRAM.
    qsplit: "none" = all DMAs on the SP HWDGE queue; "store_act" = loads
        on SP, stores on the Activation HWDGE queue; "alt" = alternate
        tiles between the two queues for both directions.
    """
    assert N_CHUNKS % blk == 0
    n_tiles = N_CHUNKS // blk
    dt = _DT[dtype]
    nc = bacc.Bacc()
    x = nc.dram_tensor("x", [SHARD, COLS], dt, kind="ExternalInput")
    w = nc.dram_tensor("weight", [COLS], dt, kind="ExternalInput")
    out = nc.dram_tensor("out", [SHARD, COLS], dt, kind="ExternalOutput")

    # DRAM view: [partition, chunk, col].
    if layout == "pn":
        xv = x.rearrange("(p n) m -> p n m", p=P)
        ov = out.rearrange("(p n) m -> p n m", p=P)
    else:
        xv = x.rearrange("(n p) m -> p n m", p=P)
        ov = out.rearrange("(n p) m -> p n m", p=P)

    with tile.TileContext(nc) as tc:
        with (
            tc.tile_pool(name="singles", bufs=1) as singles,
            tc.tile_pool(name="xs", bufs=bufs) as xpool,
        ):
            # Replicate weight across all 128 partitions.
            w_sb = singles.tile([P, COLS], dt)
            if wmode == "pbcast":
                nc.sync.dma_start(out=w_sb[:1, :], in_=w[None, :])
                nc.gpsimd.partition_broadcast(w_sb[:], w_sb[:1, :])
            else:
                nc.sync.dma_start(
                    out=w_sb[:], in_=w[None, :].to_broadcast([P, COLS])
                )

            for _ in range(reps):
                for i in range(n_tiles):
                    if qsplit == "none":
                        ld_eng, st_eng = nc.sync, nc.sync
                    elif qsplit == "store_act":
                        ld_eng, st_eng = nc.sync, nc.scalar
                    else:  # "alt"
                        ld_eng = nc.sync if i % 2 == 0 else nc.scalar
                        st_eng = nc.scalar if i % 2 == 0 else nc.sync
                    xt = xpool.tile([P, blk, COLS], dt)
                    ld_eng.dma_start(
                        out=xt[:], in_=xv[:, i * blk : (i + 1) * blk, :]
                    )
                    if fused_mul:
                        nc.vector.tensor_mul(
                            xt[:], xt[:], w_sb[:, None, :].to_broadcast([P, blk, COLS])
                        )
                    else:
                        for j in range(blk):
                            nc.vector.tensor_mul(xt[:, j, :], xt[:, j, :], w_sb[:])
                    st_eng.dma_start(
                        out=ov[:, i * blk : (i + 1) * blk, :], in_=xt[:]
                    )
    nc.finalize()
    return nc


def prep_in_maps(x: np.ndarray, weight: np.ndarray, dtype="bf16"):
    """Cast to the on-wire dtype and slice per-core row shards."""
    npdt = _NP[dtype]
    x = np.ascontiguousarray(x, dtype=np.float32).astype(npdt)
    w = np.ascontiguousarray(weight, dtype=np.float32).astype(npdt)
    return [
        {"x": x[i * SHARD : (i + 1) * SHARD], "weight": w} for i in range(N_CORES)
    ]


_nc_cache: dict = {}


def _get_nc(dtype="bf16"):
    if dtype not in _nc_cache:
        _nc_cache[dtype] = build(dtype=dtype)
    return _nc_cache[dtype]


def kernel(x: np.ndarray, weight: np.ndarray) -> np.ndarray:
    nc = _get_nc()
    in_maps = prep_in_maps(x, weight)
    # The device intermittently reports NRT_EXEC_UNIT_UNRECOVERABLE under
    # load (observed on idle-kernel runs too, not workload-dependent); it
    # clears on the next NEFF load, so retry once before giving up.
    last_err = None
    for attempt in range(2):
        try:
            res = run_bass_kernel_spmd(nc, in_maps, list(range(N_CORES))).results
            return np.concatenate([r["out"] for r in res], axis=0).astype(np.float32)
        except Exception as e:  # noqa: BLE001
            last_err = e
            time.sleep(2.0)
    raise last_err
